# revision 21
# baseline (speedup 1.0000x reference)
"""Trainium2 Bass kernel for PVT-style MHSA with spatial reduction.

Problem (hardcoded): B=4, C=384, H=W=64, NH=8 heads, HD=48, SR=2.
  q = Wq@x;  xsr = conv2x2s2(x, Wsr)+bsr;  k = (Wk@xsr + pos)*scale;  v = Wv@xsr
  attn = softmax(q^T k);  out = Wp@(v attn) + bp

Sharding: 8 cores = (batch b, query-half s).  Each core computes the full
conv/k/v for its batch (duplicated across the 2 cores of a batch) and
attention + projection for its 2048 queries.  No collectives.

Design notes (v2, rebuilt from a 347us baseline trace):
  - whole datapath is bf16 (fp32 PSUM accumulation): halves DMA, and every
    128-column weight load gets the compiler's fast-weight-load (4x) path.
    Verified numerically: end-to-end rel err ~6e-3 vs 2e-2 budget.
  - single pool scope so phase A (conv/k/q/v) overlaps phase B (attention):
    PSUM = pa(2 banks) + qk(2x2) + av(2) = 8. Emission order interleaves the
    first attention pair right after its deps so the exp stream starts ~15us
    in; remaining phase A fills PE gaps under the exp stream.
  - exp on ScalarE is the bottleneck engine (128 tiles x ~1.1us = 143us);
    everything else is shaped to hide under it.
  - attention computed transposed (keys on partitions); softmax denominator
    rides the AV matmul via an all-ones column in v^T. Ones sit at head-pair
    local columns 63 (head A) / 64 (head B) so both rowsums land on ADJACENT
    PSUM partitions 63:65 -> one [2,512] reciprocal per pair (the previous
    2x [1,512] reciprocals were 107us of DVE time).
  - x is loaded once ([128, 2(half), CT, 2048]); the query projection reads
    half 0 directly. Per-core the halves are ordered [query-half, other], and
    pos is permuted to match, so one SPMD program serves all 8 cores.
  - outputs stream out per 512-query block instead of one tail DMA.
"""

import threading

import numpy as np
import ml_dtypes

import concourse.bass as bass
import concourse.mybir as mybir
import concourse.tile as tile
from concourse import bacc
from concourse.bass import ts
from concourse.bass_utils import run_bass_kernel_spmd

B, C, H, W = 4, 384, 64, 64
NH, HD, SR = 8, 48, 2
SCALE = HD ** -0.5
Hs, Ws = H // SR, W // SR
NK = Hs * Ws            # 1024 keys
N = H * W               # 4096 queries / batch
NQ = N // 2             # 2048 queries / core
CT = C // 128           # 3 c-tiles
HP = NH // 2            # 4 head-pair tiles
NB = NQ // 512          # 4 query blocks / core
MT = NK // 128          # 8 key tiles

F32 = mybir.dt.float32
BF16 = mybir.dt.bfloat16
AF = mybir.ActivationFunctionType

DEFAULT_CFG = dict(
    pa_bufs=2, qk_bufs=2, av_bufs=2, e_bufs=3, r_bufs=3, rb_bufs=5,
    outp_bufs=2,
)


def build_program(**cfg):
    cfg = {**DEFAULT_CFG, **cfg}
    nc = bacc.Bacc(None, target_bir_lowering=False)

    xf = nc.dram_tensor("xf", [128, 2, CT, N // 2], BF16, kind="ExternalInput")
    wq = nc.dram_tensor("wq", [128, CT, 512], BF16, kind="ExternalInput")
    wk = nc.dram_tensor("wk", [128, CT, 512], BF16, kind="ExternalInput")
    wv = nc.dram_tensor("wv", [128, CT, 512], BF16, kind="ExternalInput")
    wsr = nc.dram_tensor("wsr", [128, 12, C], BF16, kind="ExternalInput")
    wp = nc.dram_tensor("wp", [128, 4, C], BF16, kind="ExternalInput")
    pos = nc.dram_tensor("pos", [128, HP, NK], BF16, kind="ExternalInput")
    bsr = nc.dram_tensor("bsr", [128, CT], F32, kind="ExternalInput")
    bp = nc.dram_tensor("bp", [128, CT], F32, kind="ExternalInput")
    out = nc.dram_tensor("out", [128, CT, NQ], F32, kind="ExternalOutput")

    with tile.TileContext(nc) as tc:
        with (
            tc.tile_pool(name="constp", bufs=1) as constp,
            tc.tile_pool(name="actp", bufs=1) as actp,
            tc.tile_pool(name="epool", bufs=cfg["e_bufs"]) as epool,
            tc.tile_pool(name="rpool", bufs=cfg["r_bufs"]) as rpool,
            tc.tile_pool(name="rbpool", bufs=cfg["rb_bufs"]) as rbpool,
            tc.tile_pool(name="outpool", bufs=cfg["outp_bufs"]) as outpool,
            tc.tile_pool(name="drp", bufs=2, space="DRAM") as drp,
            tc.tile_pool(name="paps", bufs=cfg["pa_bufs"], space="PSUM") as paps,
            tc.tile_pool(name="qkps", bufs=cfg["qk_bufs"], space="PSUM") as qkps,
            tc.tile_pool(name="avps", bufs=cfg["av_bufs"], space="PSUM") as avps,
        ):
            wq_sb = constp.tile([128, CT, 512], BF16, name="wq_sb")
            wk_sb = constp.tile([128, CT, 512], BF16, name="wk_sb")
            wv_sb = constp.tile([128, CT, 512], BF16, name="wv_sb")
            wsr_sb = constp.tile([128, 12, C], BF16, name="wsr_sb")
            wp_sb = constp.tile([128, 4, C], BF16, name="wp_sb")
            pos_sb = constp.tile([128, HP, NK], BF16, name="pos_sb")
            bsr_sb = constp.tile([128, CT], F32, name="bsr_sb")
            bp_sb = constp.tile([128, CT], F32, name="bp_sb")

            xf_sb = actp.tile([128, 2, CT, N // 2], BF16, name="xf_sb")
            xsr_sb = actp.tile([128, CT, NK], BF16, name="xsr_sb")
            q_sb = actp.tile([128, HP, NQ], BF16, name="q_sb")
            k_sb = actp.tile([128, HP, NK], BF16, name="k_sb")
            vt_sb = actp.tile([128, MT, 512], BF16, name="vt_sb")
            o_sb = actp.tile([128, HP, NQ], BF16, name="o_sb")

            # ---- input DMAs, split across the two HWDGE rings -------------
            # chunked so conv(0) can start on c-tile 0 as soon as it lands
            for ci in range(CT):
                nc.scalar.dma_start(wsr_sb[:, 4 * ci : 4 * ci + 4], wsr[:, 4 * ci : 4 * ci + 4])
            nc.scalar.dma_start(bsr_sb[:], bsr[:])
            nc.scalar.dma_start(wk_sb[:], wk[:])
            nc.scalar.dma_start(pos_sb[:, 0], pos[:, 0])
            nc.scalar.dma_start(wq_sb[:], wq[:])
            nc.scalar.dma_start(pos_sb[:, 1:], pos[:, 1:])
            nc.scalar.dma_start(wv_sb[:], wv[:])
            nc.scalar.dma_start(wp_sb[:], wp[:])
            nc.scalar.dma_start(bp_sb[:], bp[:])
            for ci in range(CT):
                nc.sync.dma_start(xf_sb[:, 0, ci], xf[:, 0, ci])
            nc.sync.dma_start(xf_sb[:, 1], xf[:, 1])

            # ---- phase A emitters ----------------------------------------
            def emit_conv(mb):
                # ci-outer accumulation: the first 4 matmuls only need
                # x c-tile 0, so conv starts while c-tiles 1,2 still stream
                for ot in range(CT):
                    p = paps.tile([128, 512], F32, name="pa", tag="pa")
                    n_mm = 0
                    for ci in range(CT):
                        for didj in range(4):
                            di, dj = didj // 2, didj % 2
                            base = xf_sb[:]
                            rhs = bass.AP(
                                tensor=base.tensor,
                                offset=base.offset
                                + mb * (CT * N // 2)
                                + ci * (N // 2)
                                + di * W
                                + dj,
                                ap=[base.ap[0], [2 * W, Hs // 2], [2, Ws]],
                            )
                            nc.tensor.matmul(
                                p[:],
                                wsr_sb[:, 4 * ci + didj, ts(ot, 128)],
                                rhs,
                                start=(n_mm == 0),
                                stop=(n_mm == 11),
                            )
                            n_mm += 1
                    nc.vector.tensor_scalar_add(
                        xsr_sb[:, ot, ts(mb, 512)], p[:], bsr_sb[:, ot : ot + 1]
                    )

            def emit_k(hp, mb):
                p = paps.tile([128, 512], F32, name="pa", tag="pa")
                for ci in range(CT):
                    nc.tensor.matmul(
                        p[:],
                        wk_sb[:, ci, ts(hp, 128)],
                        xsr_sb[:, ci, ts(mb, 512)],
                        start=(ci == 0),
                        stop=(ci == CT - 1),
                    )
                nc.vector.tensor_add(
                    k_sb[:, hp, ts(mb, 512)], p[:], pos_sb[:, hp, ts(mb, 512)]
                )

            def emit_q(ot, nb):
                p = paps.tile([128, 512], F32, name="pa", tag="pa")
                for ci in range(CT):
                    nc.tensor.matmul(
                        p[:],
                        wq_sb[:, ci, ts(ot, 128)],
                        xf_sb[:, 0, ci, ts(nb, 512)],
                        start=(ci == 0),
                        stop=(ci == CT - 1),
                    )
                nc.vector.tensor_copy(q_sb[:, ot, ts(nb, 512)], p[:])

            def emit_vt(mi):
                p = paps.tile([128, 512], F32, name="pa", tag="pa")
                for ci in range(CT):
                    nc.tensor.matmul(
                        p[:],
                        xsr_sb[:, ci, ts(mi, 128)],
                        wv_sb[:, ci, :],
                        start=(ci == 0),
                        stop=(ci == CT - 1),
                    )
                nc.vector.tensor_copy(vt_sb[:, mi, :], p[:])
                # ones columns for the softmax rowsum: head A at pair-local
                # col 63, head B at col 64 -> [[128,4],[1,2]] from offset 63
                base = vt_sb[:]
                ones_ap = bass.AP(
                    tensor=base.tensor,
                    offset=base.offset + mi * 512 + 63,
                    ap=[base.ap[0], [128, HP], [1, 2]],
                )
                nc.gpsimd.memset(ones_ap, 1.0)

            # ---- phase B emitters ----------------------------------------
            def emit_qk_exp(nb, hp, e, mi):
                qk = qkps.tile([128, 1024], F32, name="qk", tag="qk")
                nc.tensor.matmul(
                    qk[:, 0:512],
                    k_sb[0:64, hp, ts(mi, 128)],
                    q_sb[0:64, hp, ts(nb, 512)],
                    start=True,
                    stop=True,
                    tile_position=(0, 0),
                )
                nc.tensor.matmul(
                    qk[:, 512:1024],
                    k_sb[64:128, hp, ts(mi, 128)],
                    q_sb[64:128, hp, ts(nb, 512)],
                    start=True,
                    stop=True,
                    tile_position=(64, 0),
                )
                nc.scalar.activation(out=e[:, mi, :], in_=qk[:], func=AF.Exp)

            def emit_av(hp, e, oav, mi):
                nc.tensor.matmul(
                    oav[0:64, :],
                    vt_sb[:, mi, 128 * hp : 128 * hp + 64],
                    e[:, mi, 0:512],
                    start=(mi == 0),
                    stop=(mi == MT - 1),
                    tile_position=(0, 0),
                    skip_group_check=True,
                )
                nc.tensor.matmul(
                    oav[64:128, :],
                    vt_sb[:, mi, 128 * hp + 64 : 128 * (hp + 1)],
                    e[:, mi, 512:1024],
                    start=(mi == 0),
                    stop=(mi == MT - 1),
                    tile_position=(0, 64),
                    skip_group_check=True,
                )

            # normalization, batched per query block: rowsum rows of the 4
            # head-pairs are DMA-gathered to partitions 0:8 of a staging
            # tile; reciprocal is ~6.5 cyc/elem so partition-batching is the
            # only lever. Split [0:6]/[6:8] so hp0-2 normalize while hp3 is
            # still accumulating (shortens the kernel tail). oav is copied
            # out of PSUM immediately to free the bank.
            norm_state = {}

            def emit_pair_tail(nb, hp, oav):
                o_tmp = rpool.tile([128, 512], F32, name="otmp", tag="otmp", bufs=6)
                nc.vector.tensor_copy(o_tmp[:], oav[:])
                if hp == 0:
                    norm_state["stage"] = rpool.tile(
                        [128, 512], F32, name="stage", tag="stage", bufs=2
                    )
                    norm_state["otmp"] = []
                norm_state["otmp"].append(o_tmp)
                stage = norm_state["stage"]
                # DVE partition base must be 32-aligned: hp3 stages at 32:34
                rb0 = 2 * hp if hp < 3 else 32
                nc.sync.dma_start(stage[rb0 : rb0 + 2, :], o_tmp[63:65, :])

                def bcast_mul(h2, r2d):
                    rb = rbpool.tile([128, 512], F32, name="rb", tag="rb")
                    nc.sync.dma_start(
                        rb[0:64, :].unsqueeze(1),
                        r2d[2 * h2 : 2 * h2 + 1, :].partition_broadcast(64),
                    )
                    nc.sync.dma_start(
                        rb[64:128, :].unsqueeze(1),
                        r2d[2 * h2 + 1 : 2 * h2 + 2, :].partition_broadcast(64),
                    )
                    nc.vector.tensor_mul(
                        o_sb[:, h2, ts(nb, 512)], norm_state["otmp"][h2][:], rb[:]
                    )

                if hp == 2:
                    r2s = rpool.tile([128, 512], F32, name="r2s", tag="r2s", bufs=2)
                    norm_state["r2s"] = r2s
                    r2d = drp.tile([8, 512], F32, name="r2d", tag="r2d", bufs=4)
                    norm_state["r2d"] = r2d
                    nc.vector.reciprocal(out=r2s[0:6, :], in_=stage[0:6, :])
                    nc.sync.dma_start(r2d[0:6], r2s[0:6, :])
                    for h2 in range(3):
                        bcast_mul(h2, r2d)
                elif hp == 3:
                    r2s, r2d = norm_state["r2s"], norm_state["r2d"]
                    nc.vector.reciprocal(out=r2s[32:34, :], in_=stage[32:34, :])
                    nc.sync.dma_start(r2d[6:8], r2s[32:34, :])
                    bcast_mul(3, r2d)

            def emit_proj_ot(nb, ot, outp):
                p = paps.tile([128, 512], F32, name="pa", tag="pa")
                for d in range(4):
                    nc.tensor.matmul(
                        p[:],
                        wp_sb[:, d, ts(ot, 128)],
                        o_sb[:, d, ts(nb, 512)],
                        start=(d == 0),
                        stop=(d == 3),
                    )
                nc.vector.tensor_scalar_add(
                    outp[:, ot, :], p[:], bp_sb[:, ot : ot + 1]
                )
                nc.sync.dma_start(out[:, ot, ts(nb, 512)], outp[:, ot, :])

            def emit_proj(nb):
                outp = outpool.tile([128, CT, 512], F32, name="outp", tag="outp")
                for ot in range(CT):
                    emit_proj_ot(nb, ot, outp)

            # ---- emission ------------------------------------------------
            # Software-pipelined pairs: a pending pair's AV matmuls
            # interleave with a later pair's QK/exp stream (AV(X, mi) is
            # ready as soon as exp(X, mi) lands; spreading them keeps the
            # PE from bunching work and the exp stream from starving). The
            # AV lag is 2 pairs at the start (so vt + k/q prep fit in the
            # early slots) and 1 pair steady-state. Leftover phase A /
            # projection work is queued in `feed` and dripped one item per
            # exp slot so no slot overloads the PE.
            pending = []  # dicts: nb, hp, e, oav
            feed = []

            def emit_pipelined(nb, hp, navs=1, last=False):
                e = epool.tile([128, MT, 1024], BF16, name="e", tag="e")
                me = dict(nb=nb, hp=hp, e=e, oav=None)
                active = pending[:navs]
                for a in active:
                    a["oav"] = avps.tile([128, 512], F32, name="oav", tag="oav")
                if last:
                    me["oav"] = avps.tile([128, 512], F32, name="oav", tag="oav")
                for mi in range(MT):
                    emit_qk_exp(nb, hp, e, mi)
                    if feed:
                        feed.pop(0)()
                    for a in active:
                        emit_av(a["hp"], a["e"], a["oav"], mi)
                    if last and mi > 0:
                        emit_av(hp, e, me["oav"], mi - 1)
                for a in active:
                    emit_pair_tail(a["nb"], a["hp"], a["oav"])
                    pending.remove(a)
                pending.append(me)
                if last:
                    emit_av(hp, e, me["oav"], MT - 1)
                    emit_pair_tail(nb, hp, me["oav"])
                    while feed:
                        feed.pop(0)()

            # PE pre-warm: ~45 dummy matmuls during the input-DMA wait take
            # the HAM clock gate from 1.2 to 2.4 GHz before conv starts
            warm_sb = actp.tile([128, 64], BF16, name="warm_sb")
            nc.gpsimd.memset(warm_sb[:], 0.0)
            warm_ps = avps.tile([128, 512], F32, name="oav", tag="oav")
            for _ in range(45):
                nc.tensor.matmul(
                    warm_ps[0:64, 0:64], warm_sb[:], warm_sb[:], start=True, stop=True
                )

            # prefix: unblock the (nb0, hp0) exp stream ASAP
            emit_conv(0)
            emit_k(0, 0)
            emit_q(0, 0)
            e0 = epool.tile([128, MT, 1024], BF16, name="e", tag="e")
            for mi in range(4):
                emit_qk_exp(0, 0, e0, mi)
            emit_k(1, 0)
            emit_q(1, 0)
            emit_conv(1)
            emit_k(0, 1)
            for mi in range(4, MT):
                emit_qk_exp(0, 0, e0, mi)
            emit_k(1, 1)
            pending.append(dict(nb=0, hp=0, e=e0, oav=None))

            feed += [lambda: emit_k(2, 0), lambda: emit_q(2, 0)]
            feed += [(lambda m=mi: emit_vt(m)) for mi in range(6)]
            emit_pipelined(0, 1, navs=0)
            feed += [
                lambda: emit_vt(6),
                lambda: emit_vt(7),
                lambda: emit_k(2, 1),
                lambda: emit_k(3, 0),
                lambda: emit_q(3, 0),
                lambda: emit_k(3, 1),
            ]
            emit_pipelined(0, 2)  # AV(0,0)
            feed += [(lambda o=ot: emit_q(o, 1)) for ot in range(HP)]
            emit_pipelined(0, 3)  # AV(0,1)

            for nb in range(1, NB):
                for hp in range(HP):
                    last = nb == NB - 1 and hp == HP - 1
                    if hp == 1:
                        # prev block fully normalized once tail(nb-1,3)
                        # lands; drip its projection one c-tile per slot
                        outp = outpool.tile(
                            [128, CT, 512], F32, name="outp", tag="outp"
                        )
                        feed += [
                            (lambda o=ot, b=nb - 1, t=outp: emit_proj_ot(b, o, t))
                            for ot in range(CT)
                        ]
                    if hp == 2 and nb < NB - 1:
                        feed += [
                            (lambda o=ot, b=nb + 1: emit_q(o, b)) for ot in range(HP)
                        ]
                    navs = 2 if (nb == 1 and hp == 0) else 1
                    emit_pipelined(nb, hp, navs=navs, last=last)
            emit_proj(NB - 1)

    nc.compile()
    return nc


def _headcol(h, j):
    """v^T / proj channel placement: head h channel j -> flat column.
    Even heads: cols 0..47 of their 64-block (ones ride at 63); odd heads:
    cols 1..48 (ones at 0) so both rowsums land on adjacent partitions."""
    return 64 * h + (j if h % 2 == 0 else j + 1)


_BF = ml_dtypes.bfloat16


def _ctile(w):
    """[C, F] -> [128, CT, F] (partition-major c-tiles)."""
    return np.ascontiguousarray(w.reshape(CT, 128, -1).transpose(1, 0, 2))


def prep_inputs(inputs):
    x = np.asarray(inputs["x"], np.float32)
    Wq = np.asarray(inputs["Wq"], np.float32)
    Wk = np.asarray(inputs["Wk"], np.float32)
    Wv = np.asarray(inputs["Wv"], np.float32)
    Wsr = np.asarray(inputs["Wsr"], np.float32)
    bsr = np.asarray(inputs["bsr"], np.float32)
    Wp = np.asarray(inputs["Wp"], np.float32)
    bp = np.asarray(inputs["bp"], np.float32)
    rel_h = np.asarray(inputs["rel_h"], np.float32)
    rel_w = np.asarray(inputs["rel_w"], np.float32)

    def pad_cols(w):
        """[C, C] -> [C, 512]: col 64h+j = w[48h+j, :] (j < 48)."""
        wt = np.zeros((C, NH * 64), np.float32)
        for h in range(NH):
            wt[:, 64 * h : 64 * h + HD] = w[HD * h : HD * (h + 1), :].T
        return wt

    wq_t = _ctile(pad_cols(Wq)).astype(_BF)
    wk_t = _ctile(pad_cols(Wk) * SCALE).astype(_BF)
    wv_pad = np.zeros((C, NH * 64), np.float32)
    for h in range(NH):
        for j in range(HD):
            wv_pad[:, _headcol(h, j)] = Wv[HD * h + j, :]
    wv_t = _ctile(wv_pad).astype(_BF)
    # conv weights: [128, 12, C] with slice index ci*4 + didj (ci-major so
    # conv can start after x c-tile 0 arrives)
    wsr_t = np.ascontiguousarray(
        Wsr.transpose(1, 2, 3, 0)
        .reshape(CT, 128, 4, C)
        .transpose(1, 0, 2, 3)
        .reshape(128, 12, C)
    ).astype(_BF)
    # proj weights: row headcol(h, j) = Wp[:, 48h+j]
    wp_t = np.zeros((NH * 64, C), np.float32)
    for h in range(NH):
        for j in range(HD):
            wp_t[_headcol(h, j), :] = Wp[:, HD * h + j]
    wp_t = np.ascontiguousarray(wp_t.reshape(4, 128, C).transpose(1, 0, 2)).astype(_BF)
    # positional bias, pre-scaled, padded to 64-channel heads -> [128, HP, NK]
    pos_flat = (rel_h + rel_w).reshape(NH, HD, NK).astype(np.float32) * SCALE
    pos_t = np.zeros((NH * 64, NK), np.float32)
    for h in range(NH):
        pos_t[64 * h : 64 * h + HD, :] = pos_flat[h]
    pos_t = np.ascontiguousarray(pos_t.reshape(HP, 128, NK).transpose(1, 0, 2))
    bsr_t = np.ascontiguousarray(bsr.reshape(CT, 128).T)
    bp_t = np.ascontiguousarray(bp.reshape(CT, 128).T)

    in_maps = []
    for core in range(8):
        b, s = core // 2, core % 2
        xb = x[b].reshape(C, N)
        # halves ordered [query-half, other-half] so q reads slot 0; keys
        # are correspondingly permuted per-core, with pos permuted to match
        xf_t = np.ascontiguousarray(
            xb.reshape(CT, 128, 2, N // 2).transpose(1, 2, 0, 3)
        )
        if s == 1:
            xf_t = np.ascontiguousarray(xf_t[:, ::-1])
            pos_c = np.ascontiguousarray(
                np.concatenate([pos_t[:, :, 512:], pos_t[:, :, :512]], axis=2)
            )
        else:
            pos_c = pos_t
        in_maps.append(
            {
                "xf": xf_t.astype(_BF),
                "wq": wq_t,
                "wk": wk_t,
                "wv": wv_t,
                "wsr": wsr_t,
                "wp": wp_t,
                "pos": pos_c.astype(_BF),
                "bsr": bsr_t,
                "bp": bp_t,
            }
        )
    return in_maps


def assemble_output(results):
    out = np.empty((B, C, N), np.float32)
    for core in range(8):
        b, s = core // 2, core % 2
        out[b, :, s * NQ : (s + 1) * NQ] = (
            results[core]["out"].transpose(1, 0, 2).reshape(C, NQ)
        )
    return out.reshape(B, C, H, W)


_cache = threading.Lock()
_program = None


def get_program():
    global _program
    with _cache:
        if _program is None:
            _program = build_program()
    return _program


def run(inputs, **kwargs):
    nc = get_program()
    in_maps = prep_inputs(inputs)
    res = run_bass_kernel_spmd(nc, in_maps, core_ids=list(range(8)), **kwargs)
    return assemble_output(res.results), res


def kernel(**inputs):
    out, _ = run(inputs)
    return out
